# revision 15
# baseline (speedup 1.0000x reference)
"""GCN (2-layer GraphConv + mean-pool + linear head) on 8 Trainium2 NeuronCores.

Strategy (per sharding hint: partition nodes/edges, replicate weights,
all-reduce pooled sums):
  - Nodes are partitioned contiguously: core c owns nodes [c*N/8, (c+1)*N/8).
  - Edges are assigned to the core owning their dst.
  - The node-feature table is replicated in each core's HBM; x[src] rows are
    fetched with SWDGE dma_gather (int16 indices -> table split into 32768-row
    banks; edges grouped by bank).
  - segment_sum(x[src], dst) is computed as a sequence of PE matmuls:
    gathered payload chunks [128e, 128f] are contracted against on-the-fly
    0/1 indicator matrices [128e, 128n] (built on DVE via tensor_scalar
    is_equal against an iota row), accumulating feature-major aggregates in
    PSUM. out_norm[src] is folded into the indicator values (layer 1) or the
    gathered table itself (layer 2); in_norm[dst] is folded into the epilogue
    activation scale. This is deterministic (dma_scatter_add races on
    duplicate indices and cannot be used for degree-16 graphs).
  - Layer-1 output (the layer-2 gather table) is exchanged with an AllGather;
    per-graph pooled sums are combined with an AllReduce; the tiny classifier
    head runs replicated on every core.
"""
import sys

sys.path.insert(0, "/opt/trn_rl_repo")

import numpy as np

NCORES = 8
BANKW = 32768          # dma_gather int16 index limit -> table bank rows
BLK = 128              # dst-node block (matmul N)
SBB = 4                # blocks per superblock (PSUM accum tile = 4 blocks)
NUM_GRAPHS = 128


def _ceil(a, b):
    return -(-a // b)


def _build_structure(src, dst, n_nodes, own, slice_rows, nblk):
    """Host-side edge partitioning. Returns per-core packed arrays + the
    (uniform across cores) chunk structure."""
    nsb = _ceil(nblk, SBB)
    t_all = (src // own) * slice_rows + (src % own)  # table row of src
    ntab = NCORES * slice_rows
    nbanks = _ceil(ntab, BANKW)
    core_of = dst // own

    per_core = []
    counts = np.zeros((NCORES, nblk * nbanks), np.int64)
    for c in range(NCORES):
        m = core_of == c
        s_t = t_all[m]
        dl = (dst[m] - c * own).astype(np.int64)
        blk = dl >> 7
        bank = s_t >> 15
        grp = blk * nbanks + bank
        order = np.argsort(grp, kind="stable")
        per_core.append((s_t[order], dl[order], grp[order]))
        counts[c] = np.bincount(grp, minlength=nblk * nbanks)

    nchunks = _ceil(np.maximum(counts.max(axis=0), 0), 128)  # [nblk*nbanks]
    # chunk_meta in program order: for sb, for bank, for blk in sb, chunks
    chunk_meta = []   # (blk, bank)
    calls = []        # (sb, bank, chunk_start, nch_call)
    for sb in range(nsb):
        blks = range(sb * SBB, min((sb + 1) * SBB, nblk))
        for bank in range(nbanks):
            k0 = len(chunk_meta)
            for blk in blks:
                chunk_meta.extend([(blk, bank)] * int(nchunks[blk * nbanks + bank]))
            if len(chunk_meta) > k0:
                calls.append((sb, bank, k0, len(chunk_meta) - k0))
    nch_tot = len(chunk_meta)
    eproc = nch_tot * 128

    # position ranges per (blk, bank) group, in chunk_meta order
    grp_pos = {}
    pos = 0
    for k, (blk, bank) in enumerate(chunk_meta):
        g = blk * nbanks + bank
        if g not in grp_pos:
            grp_pos[g] = pos
        pos += 128

    return {
        "nsb": nsb, "nbanks": nbanks, "ntab": ntab, "nchunks": nchunks,
        "chunk_meta": chunk_meta, "calls": calls, "nch_tot": nch_tot,
        "eproc": eproc, "grp_pos": grp_pos, "per_core": per_core,
        "counts": counts,
    }


def _pack_core(st, c, out_norm_t, nblk):
    """Build gidx (wrapped int16), dstoff/onorm column-packed arrays for core c."""
    nbanks = st["nbanks"]
    eproc = st["eproc"]
    s_t, dl, grp = st["per_core"][c]
    cnt = st["counts"][c]

    gidx = np.zeros(eproc, np.int64)
    dstoff = np.full(eproc, -1.0, np.float32)
    onorm = np.zeros(eproc, np.float32)

    gstart = np.zeros(nblk * nbanks + 1, np.int64)
    np.cumsum(cnt, out=gstart[1:])
    for g, p0 in st["grp_pos"].items():
        ne = int(cnt[g])
        if ne == 0:
            continue
        e0 = int(gstart[g])
        blk = g // nbanks
        bank = g % nbanks
        sl = slice(p0, p0 + ne)
        gidx[sl] = s_t[e0:e0 + ne] - bank * BANKW
        dstoff[sl] = (dl[e0:e0 + ne] - blk * 128).astype(np.float32)
        onorm[sl] = out_norm_t[s_t[e0:e0 + ne]]

    gw = gidx.astype(np.int16).reshape(-1, 16).T          # [16, eproc/16]
    gw = np.ascontiguousarray(np.tile(gw, (8, 1)))        # [128, eproc/16]
    dstoff_p = np.ascontiguousarray(dstoff.reshape(-1, 128).T)  # [128, nch]
    onorm_p = np.ascontiguousarray(onorm.reshape(-1, 128).T)
    return gw, dstoff_p, onorm_p


def _build_program(st, nblk, slice_rows, dfeat, hfeat, pfeat, ncls, with_b1, with_b2, with_bc,
                   debug_dumps=False):
    import concourse.bacc as bacc
    import concourse.mybir as mybir
    import concourse.tile as tile

    f32 = mybir.dt.float32
    i16 = mybir.dt.int16
    Alu = mybir.AluOpType
    Act = mybir.ActivationFunctionType

    nsb, nbanks, ntab = st["nsb"], st["nbanks"], st["ntab"]
    nch_tot, eproc = st["nch_tot"], st["eproc"]
    calls, chunk_meta, nchunks = st["calls"], st["chunk_meta"], st["nchunks"]
    cmax = max(nc_ for (_, _, _, nc_) in calls)
    rg = [list(range(NCORES))]

    nc = bacc.Bacc("TRN2", target_bir_lowering=False, debug=False, num_devices=NCORES)

    # ---- I/O ----
    h_t = nc.dram_tensor("h_t", [ntab, dfeat], f32, kind="ExternalInput")
    gidx_in = nc.dram_tensor("gidx", [128, eproc // 16], i16, kind="ExternalInput")
    dstoff_in = nc.dram_tensor("dstoff", [128, nch_tot], f32, kind="ExternalInput")
    onorm_in = nc.dram_tensor("onorm", [128, nch_tot], f32, kind="ExternalInput")
    sio1_in = nc.dram_tensor("sio1", [128, nblk], f32, kind="ExternalInput")
    sin2_in = nc.dram_tensor("sin2", [128, nblk], f32, kind="ExternalInput")
    iota_in = nc.dram_tensor("iota_b", [128, 128], f32, kind="ExternalInput")
    w1_in = nc.dram_tensor("w1", [dfeat, hfeat], f32, kind="ExternalInput")
    w2_in = nc.dram_tensor("w2", [hfeat, hfeat], f32, kind="ExternalInput")
    wca_in = nc.dram_tensor("wca", [hfeat, ncls], f32, kind="ExternalInput")
    wcb_in = nc.dram_tensor("wcb", [pfeat, ncls], f32, kind="ExternalInput")
    permt_in = nc.dram_tensor("permt", [pfeat, NUM_GRAPHS], f32, kind="ExternalInput")
    pind_in = nc.dram_tensor("pind", [slice_rows, NUM_GRAPHS], f32, kind="ExternalInput")
    b1_in = nc.dram_tensor("b1b", [128, hfeat], f32, kind="ExternalInput") if with_b1 else None
    b2_in = nc.dram_tensor("b2b", [128, hfeat], f32, kind="ExternalInput") if with_b2 else None
    bc_in = nc.dram_tensor("bcc", [128, 1], f32, kind="ExternalInput") if with_bc else None
    out_ext = nc.dram_tensor("outT", [ncls, NUM_GRAPHS], f32, kind="ExternalOutput")

    dbg_x1s = dbg_t2 = dbg_hg = None
    if debug_dumps:
        dbg_x1s = nc.dram_tensor("dbg_x1s", [slice_rows, hfeat], f32, kind="ExternalOutput")
        dbg_t2 = nc.dram_tensor("dbg_t2", [ntab, hfeat], f32, kind="ExternalOutput")
        dbg_hg = nc.dram_tensor("dbg_hg", [hfeat, NUM_GRAPHS], f32, kind="ExternalOutput")

    # ---- internal DRAM ----
    x1s = nc.dram_tensor("x1s", [slice_rows, hfeat], f32)                     # AG input
    t2 = nc.dram_tensor("t2", [ntab, hfeat], f32, addr_space="Shared")        # AG output
    ar_in = nc.dram_tensor("ar_in", [hfeat, NUM_GRAPHS], f32)
    ar_out = nc.dram_tensor("ar_out", [hfeat, NUM_GRAPHS], f32, addr_space="Shared")

    x1s_v = x1s[:].rearrange("(c p) d -> p c d", p=128)   # [128, nblk, hfeat]
    pind_v = pind_in[:].rearrange("(c p) g -> p c g", p=128)

    with tile.TileContext(nc) as tc:
        with (
            tc.tile_pool(name="const", bufs=1) as cpool,
            tc.tile_pool(name="payload", bufs=3) as ppool,
            tc.tile_pool(name="ind", bufs=8) as ipool,
            tc.tile_pool(name="epi", bufs=3) as epool,
            tc.tile_pool(name="accum", bufs=5, space="PSUM") as apsum,
            tc.tile_pool(name="ypsum", bufs=2, space="PSUM") as ypsum,
            tc.tile_pool(name="gpsum", bufs=1, space="PSUM") as gpsum,
        ):
            # ---- constants ----
            gidx_t = cpool.tile([128, eproc // 16], i16)
            dstoff_t = cpool.tile([128, nch_tot], f32)
            onorm_t = cpool.tile([128, nch_tot], f32)
            sio1_t = cpool.tile([128, nblk], f32)
            sin2_t = cpool.tile([128, nblk], f32)
            iota_t = cpool.tile([128, 128], f32)
            w1_t = cpool.tile([dfeat, hfeat], f32)
            w2_t = cpool.tile([hfeat, hfeat], f32)
            wca_t = cpool.tile([hfeat, ncls], f32)
            wcb_t = cpool.tile([pfeat, ncls], f32)
            permt_t = cpool.tile([pfeat, NUM_GRAPHS], f32)
            for t_, i_ in [(gidx_t, gidx_in), (dstoff_t, dstoff_in), (onorm_t, onorm_in),
                           (sio1_t, sio1_in), (sin2_t, sin2_in), (iota_t, iota_in),
                           (w1_t, w1_in), (w2_t, w2_in), (wca_t, wca_in),
                           (wcb_t, wcb_in), (permt_t, permt_in)]:
                nc.sync.dma_start(t_[:], i_[:])
            b1_t = b2_t = bc_t = None
            if with_b1:
                b1_t = cpool.tile([128, hfeat], f32)
                nc.sync.dma_start(b1_t[:], b1_in[:])
            if with_b2:
                b2_t = cpool.tile([128, hfeat], f32)
                nc.sync.dma_start(b2_t[:], b2_in[:])
            if with_bc:
                bc_t = cpool.tile([128, 1], f32)
                nc.sync.dma_start(bc_t[:], bc_in[:])

            hg_ps = gpsum.tile([hfeat, NUM_GRAPHS], f32)

            def edge_layer(layer):
                """One GCN layer: gather + indicator matmuls + epilogue."""
                table = h_t if layer == 1 else t2
                nf = dfeat if layer == 1 else hfeat
                # first/last chunk bookkeeping per blk
                blk_total = np.zeros(nblk, np.int64)
                for (blk, bank) in chunk_meta:
                    blk_total[blk] += 1
                blk_seen = np.zeros(nblk, np.int64)

                # calls grouped by sb
                sb_calls = {}
                for call in calls:
                    sb_calls.setdefault(call[0], []).append(call)

                for sb in range(nsb):
                    blks = list(range(sb * SBB, min((sb + 1) * SBB, nblk)))
                    # one PSUM tile per block: a bank must host only one
                    # accumulation group at a time (start=True resets the
                    # whole bank's has_written state)
                    accs = [apsum.tile([nf, 128], f32, tag="acc", name=f"acc_{layer}_{sb}_{i}")
                            for i in range(len(blks))]
                    for b4, blk in enumerate(blks):
                        if blk_total[blk] == 0:
                            nc.vector.memset(accs[b4][:], 0.0)
                    for (_, bank, k0, nch) in sb_calls.get(sb, []):
                        pt = ppool.tile([128, cmax, 128], f32, tag="pay")
                        lo = bank * BANKW
                        hi = min(lo + BANKW, ntab)
                        nc.gpsimd.dma_gather(
                            pt[:, :nch, :], table[lo:hi, :],
                            gidx_t[:, 8 * k0: 8 * (k0 + nch)],
                            128 * nch, 128 * nch, nf, single_packet=False)
                        for j in range(nch):
                            k = k0 + j
                            blk = chunk_meta[k][0]
                            b4 = blk - sb * SBB
                            ind = ipool.tile([128, 128], f32, tag="ind")
                            if layer == 1:
                                nc.vector.tensor_scalar(
                                    ind[:], iota_t[:],
                                    dstoff_t[:, k:k + 1], onorm_t[:, k:k + 1],
                                    Alu.is_equal, Alu.mult)
                            else:
                                nc.vector.tensor_scalar(
                                    ind[:], iota_t[:],
                                    dstoff_t[:, k:k + 1], None, Alu.is_equal)
                            blk_seen[blk] += 1
                            nc.tensor.matmul(
                                accs[b4][:], pt[:, j, :], ind[:],
                                start=bool(blk_seen[blk] == 1),
                                stop=bool(blk_seen[blk] == blk_total[blk]),
                                skip_group_check=True)
                    # ---- epilogue for this superblock ----
                    aggT = epool.tile([nf, SBB * 128], f32, tag="aggT")
                    aggTv = aggT[:].rearrange("p (b n) -> p b n", b=SBB)
                    for b4 in range(len(blks)):
                        nc.vector.tensor_copy(aggTv[:, b4, :], accs[b4][:])
                    if layer == 1:
                        xt = epool.tile([128, SBB, hfeat], f32, tag="xt")
                        for b4, blk in enumerate(blks):
                            y = ypsum.tile([128, hfeat], f32, tag="y")
                            nc.tensor.matmul(y[:], aggTv[:, b4, :], w1_t[:],
                                             start=True, stop=True, skip_group_check=True)
                            if with_b1:
                                # x = relu(in_norm*y + b1) * out_norm
                                # (sio1 holds out_norm in this mode, sin2 = in_norm)
                                v = epool.tile([128, hfeat], f32, tag="v")
                                nc.vector.tensor_scalar(v[:], y[:], sin2_t[:, blk:blk + 1],
                                                        None, Alu.mult)
                                nc.vector.tensor_tensor(v[:], v[:], b1_t[:], Alu.add)
                                nc.scalar.activation(xt[:, b4, :], v[:], Act.Relu,
                                                     scale=sio1_t[:, blk:blk + 1])
                            else:
                                # x = relu(y) * in*out = relu(y * (in*out))
                                nc.scalar.activation(xt[:, b4, :], y[:], Act.Relu,
                                                     scale=sio1_t[:, blk:blk + 1])
                        nc.sync.dma_start(x1s_v[:, sb * SBB: sb * SBB + len(blks), :],
                                          xt[:, :len(blks), :])
                    else:
                        for b4, blk in enumerate(blks):
                            y = ypsum.tile([128, hfeat], f32, tag="y")
                            nc.tensor.matmul(y[:], aggTv[:, b4, :], w2_t[:],
                                             start=True, stop=True, skip_group_check=True)
                            x2 = epool.tile([128, hfeat], f32, tag="x2")
                            nc.scalar.activation(x2[:], y[:], Act.Copy,
                                                 scale=sin2_t[:, blk:blk + 1])
                            if with_b2:
                                nc.vector.tensor_tensor(x2[:], x2[:], b2_t[:], Alu.add)
                            pi = epool.tile([128, NUM_GRAPHS], f32, tag="pi")
                            nc.sync.dma_start(pi[:], pind_v[:, blk, :])
                            nc.tensor.matmul(hg_ps[:], x2[:], pi[:],
                                             start=bool(blk == 0),
                                             stop=bool(blk == nblk - 1),
                                             skip_group_check=True)

            # NOTE on layer-1 bias: for with_b1 the sio1 scale on `u` above is
            # unused scratch; the correct path uses sin2 (=in_norm) + b1 + relu
            # with sio... see host: sio1 = in*out for no-bias, out-only for bias.

            edge_layer(1)
            nc.gpsimd.collective_compute(
                "AllGather", mybir.AluOpType.bypass, replica_groups=rg,
                ins=[x1s[:]], outs=[t2[:]])
            edge_layer(2)
            if debug_dumps:
                nc.sync.dma_start(dbg_x1s[:], x1s[:])
                nc.sync.dma_start(dbg_t2[:], t2[:])

            # ---- pooled sums all-reduce + head ----
            hg_sb = epool.tile([hfeat, NUM_GRAPHS], f32, tag="hg")
            nc.vector.tensor_copy(hg_sb[:], hg_ps[:])
            nc.sync.dma_start(ar_in[:], hg_sb[:])
            nc.gpsimd.collective_compute(
                "AllReduce", mybir.AluOpType.add, replica_groups=rg,
                ins=[ar_in[:]], outs=[ar_out[:]])
            hg_all = epool.tile([hfeat, NUM_GRAPHS], f32, tag="hga")
            nc.sync.dma_start(hg_all[:], ar_out[:])
            if debug_dumps:
                nc.sync.dma_start(dbg_hg[:], ar_in[:])

            o_ps = ypsum.tile([128, NUM_GRAPHS], f32, tag="y")
            nc.tensor.matmul(o_ps[:ncls, :], wca_t[:], hg_all[:],
                             start=True, stop=False, skip_group_check=True)
            nc.tensor.matmul(o_ps[:ncls, :], wcb_t[:], permt_t[:],
                             start=False, stop=True, skip_group_check=True)
            ob = epool.tile([128, NUM_GRAPHS], f32, tag="ob")
            if with_bc:
                nc.vector.tensor_scalar(ob[:ncls, :], o_ps[:ncls, :],
                                        bc_t[:ncls, :], None, Alu.add)
            else:
                nc.vector.tensor_copy(ob[:ncls, :], o_ps[:ncls, :])
            nc.sync.dma_start(out_ext[:], ob[:ncls, :])

    nc.compile()
    return nc


def kernel(h, perm_features, W1, b1, W2, b2, Wc, bc, src, dst, graph_ids):
    from concourse.bass_utils import run_bass_kernel_spmd

    h = np.ascontiguousarray(np.asarray(h, dtype=np.float32))
    perm_features = np.asarray(perm_features, dtype=np.float32)
    W1 = np.asarray(W1, dtype=np.float32)
    W2 = np.asarray(W2, dtype=np.float32)
    Wc = np.asarray(Wc, dtype=np.float32)
    b1 = np.asarray(b1, dtype=np.float32)
    b2 = np.asarray(b2, dtype=np.float32)
    bc = np.asarray(bc, dtype=np.float32)
    src = np.asarray(src).astype(np.int64)
    dst = np.asarray(dst).astype(np.int64)
    graph_ids = np.asarray(graph_ids).astype(np.int64)

    n_nodes, dfeat = h.shape
    hfeat = W1.shape[1]
    pfeat = perm_features.shape[1]
    ncls = Wc.shape[1]
    n_edges = src.shape[0]
    own = _ceil(n_nodes, NCORES)
    nblk = _ceil(own, 128)
    slice_rows = nblk * 128
    ntab = NCORES * slice_rows

    # ---- degrees / norms (host: index histograms, part of graph partitioning) ----
    out_deg = np.bincount(src, minlength=n_nodes).astype(np.float64)
    in_deg = np.bincount(dst, minlength=n_nodes).astype(np.float64)
    out_norm = (1.0 / np.sqrt(np.maximum(out_deg, 1.0))).astype(np.float32)
    in_norm = (1.0 / np.sqrt(np.maximum(in_deg, 1.0))).astype(np.float32)

    # node table in t-order (core-major with per-core pad to slice_rows)
    h_tab = np.zeros((ntab, dfeat), np.float32)
    out_norm_t = np.zeros(ntab, np.float32)
    in_norm_t = np.ones(ntab, np.float32)
    for c in range(NCORES):
        lo, hi = c * own, min((c + 1) * own, n_nodes)
        h_tab[c * slice_rows: c * slice_rows + (hi - lo)] = h[lo:hi]
        out_norm_t[c * slice_rows: c * slice_rows + (hi - lo)] = out_norm[lo:hi]
        in_norm_t[c * slice_rows: c * slice_rows + (hi - lo)] = in_norm[lo:hi]

    st = _build_structure(src, dst, n_nodes, own, slice_rows, nblk)

    with_b1 = bool(np.any(b1))
    with_b2 = bool(np.any(b2))
    with_bc = bool(np.any(bc))

    # per-block scale columns (per core)
    counts = np.bincount(graph_ids, minlength=NUM_GRAPHS).astype(np.float64)
    invcnt = (1.0 / np.maximum(counts, 1.0)).astype(np.float32)

    iota_b = np.tile(np.arange(128, dtype=np.float32), (128, 1))
    permt = np.ascontiguousarray(perm_features.T)

    in_maps = []
    for c in range(NCORES):
        gw, dstoff_p, onorm_p = _pack_core(st, c, out_norm_t, nblk)
        sl = slice(c * slice_rows, (c + 1) * slice_rows)
        s_in = in_norm_t[sl].reshape(nblk, 128).T.copy()     # [128, nblk]
        s_out = np.where(out_norm_t[sl] > 0, out_norm_t[sl], 1.0) \
            .reshape(nblk, 128).T.copy()
        sio1 = np.ascontiguousarray(s_in * s_out) if not with_b1 else np.ascontiguousarray(s_out)
        # pooling indicator with 1/count folded in
        gsl = graph_ids[c * own: min((c + 1) * own, n_nodes)]
        pind = np.zeros((slice_rows, NUM_GRAPHS), np.float32)
        pind[np.arange(gsl.shape[0]), gsl] = invcnt[gsl]
        m = {
            "h_t": h_tab, "gidx": gw, "dstoff": dstoff_p, "onorm": onorm_p,
            "sio1": sio1, "sin2": np.ascontiguousarray(s_in), "iota_b": iota_b,
            "w1": W1, "w2": W2, "wca": np.ascontiguousarray(Wc[:hfeat]),
            "wcb": np.ascontiguousarray(Wc[hfeat:]), "permt": permt,
            "pind": pind,
        }
        if with_b1:
            m["b1b"] = np.tile(b1, (128, 1))
        if with_b2:
            m["b2b"] = np.tile(b2, (128, 1))
        if with_bc:
            m["bcc"] = np.pad(bc, (0, 128 - ncls)).reshape(128, 1)
        in_maps.append(m)

    import os
    nc = _build_program(st, nblk, slice_rows, dfeat, hfeat, pfeat, ncls,
                        with_b1, with_b2, with_bc,
                        debug_dumps=os.environ.get("KERNEL_DEBUG") == "1")
    if os.environ.get("KERNEL_TIME") == "1":
        out0, times = _timed_run(nc, in_maps)
        kernel.last_times = times
        return np.ascontiguousarray(out0["outT"].T)
    r = run_bass_kernel_spmd(nc, in_maps, list(range(NCORES)))
    kernel.last_result = r
    return np.ascontiguousarray(r.results[0]["outT"].T)


def _timed_run(nc, in_maps, iters=6):
    """Run the program on 8 cores with a persistent jit (no donation) and
    time repeated executions."""
    import time

    import jax
    import numpy as np
    from jax.experimental.shard_map import shard_map
    from jax.sharding import Mesh, PartitionSpec

    from concourse import bass2jax, mybir

    bass2jax.install_neuronx_cc_hook()
    n_cores = NCORES
    partition_name = nc.partition_id_tensor.name if nc.partition_id_tensor else None
    in_names, out_names, out_avals, zero_outs = [], [], [], []
    for alloc in nc.m.functions[0].allocations:
        if not isinstance(alloc, mybir.MemoryLocationSet):
            continue
        name = alloc.memorylocations[0].name
        if alloc.kind == "ExternalInput":
            if name != partition_name:
                in_names.append(name)
        elif alloc.kind == "ExternalOutput":
            out_names.append(name)
            shape = tuple(alloc.tensor_shape)
            dtype = mybir.dt.np(alloc.dtype)
            out_avals.append(jax.core.ShapedArray(shape, dtype))
            zero_outs.append(np.zeros(shape, dtype))
    n_params = len(in_names)
    all_in_names = in_names + out_names + ([partition_name] if partition_name else [])

    def _body(*args):
        operands = list(args)
        if partition_name is not None:
            operands.append(bass2jax.partition_id_tensor())
        outs = bass2jax._bass_exec_p.bind(
            *operands,
            out_avals=tuple(out_avals),
            in_names=tuple(all_in_names),
            out_names=tuple(out_names),
            lowering_input_output_aliases=(),
            sim_require_finite=True,
            sim_require_nnan=True,
            nc=nc,
        )
        return tuple(outs)

    devices = jax.devices()[:n_cores]
    mesh = Mesh(np.asarray(devices), ("core",))
    in_specs = (PartitionSpec("core"),) * (n_params + len(out_names))
    out_specs = (PartitionSpec("core"),) * len(out_names)
    sharded = jax.jit(shard_map(_body, mesh=mesh, in_specs=in_specs,
                                out_specs=out_specs, check_rep=False),
                      keep_unused=True)
    concat_in = [np.concatenate([np.asarray(in_maps[c][nm]) for c in range(n_cores)], axis=0)
                 for nm in in_names]
    concat_zeros = [np.zeros((n_cores * z.shape[0], *z.shape[1:]), z.dtype)
                    for z in zero_outs]
    # stage on device once
    dev_in = [jax.device_put(a) for a in concat_in]
    dev_z = [jax.device_put(a) for a in concat_zeros]
    out = sharded(*dev_in, *dev_z)
    jax.block_until_ready(out)
    times = []
    for _ in range(iters):
        t0 = time.perf_counter()
        out = sharded(*dev_in, *dev_z)
        jax.block_until_ready(out)
        times.append(time.perf_counter() - t0)
    out0 = {nm: np.asarray(out[i]).reshape(n_cores, *out_avals[i].shape)[0]
            for i, nm in enumerate(out_names)}
    return out0, times


# revision 35
# speedup vs baseline: 1.2117x; 1.2117x over previous
"""GCN (2-layer GraphConv + mean-pool + linear head) on 8 Trainium2 NeuronCores.

Strategy (per sharding hint: partition nodes/edges, replicate weights,
all-reduce pooled sums):
  - Nodes are partitioned contiguously: core c owns nodes [c*N/8, (c+1)*N/8).
  - Edges are assigned to the core owning their dst.
  - The node-feature table is replicated in each core's HBM; x[src] rows are
    fetched with SWDGE dma_gather (int16 indices -> table split into 32768-row
    banks; edges grouped by bank).
  - segment_sum(x[src], dst) is computed as a sequence of PE matmuls:
    gathered payload chunks [128e, 128f] are contracted against on-the-fly
    0/1 indicator matrices [128e, 128n] (built on DVE via tensor_scalar
    is_equal against an iota row), accumulating feature-major aggregates in
    PSUM. out_norm[src] is folded into the indicator values (layer 1) or the
    gathered table itself (layer 2); in_norm[dst] is folded into the epilogue
    activation scale. This is deterministic (dma_scatter_add races on
    duplicate indices and cannot be used for degree-16 graphs).
  - Layer-1 output (the layer-2 gather table) is exchanged with an AllGather;
    per-graph pooled sums are combined with an AllReduce; the tiny classifier
    head runs replicated on every core.
"""
import sys

sys.path.insert(0, "/opt/trn_rl_repo")

import numpy as np

NCORES = 8
BANKW = 32768          # dma_gather int16 index limit -> table bank rows
BLK = 128              # dst-node block (matmul N)
SBB = 4                # blocks per superblock (PSUM accum tile = 4 blocks)
NUM_GRAPHS = 128


def _ceil(a, b):
    return -(-a // b)


def _build_structure(src, dst, n_nodes, own, slice_rows, nblk):
    """Host-side edge partitioning. Returns per-core packed arrays + the
    (uniform across cores) chunk structure."""
    nsb = _ceil(nblk, SBB)
    t_all = (src // own) * slice_rows + (src % own)  # table row of src
    ntab = NCORES * slice_rows
    nbanks = _ceil(ntab, BANKW)
    core_of = dst // own

    per_core = []
    counts = np.zeros((NCORES, nblk * nbanks), np.int64)
    for c in range(NCORES):
        m = core_of == c
        s_t = t_all[m]
        dl = (dst[m] - c * own).astype(np.int64)
        blk = dl >> 7
        bank = s_t >> 15
        grp = blk * nbanks + bank
        order = np.argsort(grp, kind="stable")
        per_core.append((s_t[order], dl[order], grp[order]))
        counts[c] = np.bincount(grp, minlength=nblk * nbanks)

    nchunks = _ceil(np.maximum(counts.max(axis=0), 0), 128)  # [nblk*nbanks]
    # chunk_meta in program order: for sb, for bank, for blk in sb, chunks
    chunk_meta = []   # (blk, bank)
    calls = []        # (sb, bank, chunk_start, nch_call)
    for sb in range(nsb):
        blks = range(sb * SBB, min((sb + 1) * SBB, nblk))
        for bank in range(nbanks):
            k0 = len(chunk_meta)
            for blk in blks:
                chunk_meta.extend([(blk, bank)] * int(nchunks[blk * nbanks + bank]))
            if len(chunk_meta) > k0:
                calls.append((sb, bank, k0, len(chunk_meta) - k0))
    nch_tot = len(chunk_meta)
    eproc = nch_tot * 128

    # position ranges per (blk, bank) group, in chunk_meta order
    grp_pos = {}
    pos = 0
    for k, (blk, bank) in enumerate(chunk_meta):
        g = blk * nbanks + bank
        if g not in grp_pos:
            grp_pos[g] = pos
        pos += 128

    return {
        "nsb": nsb, "nbanks": nbanks, "ntab": ntab, "nchunks": nchunks,
        "chunk_meta": chunk_meta, "calls": calls, "nch_tot": nch_tot,
        "eproc": eproc, "grp_pos": grp_pos, "per_core": per_core,
        "counts": counts,
    }


def _pack_core(st, c, out_norm_t, nblk):
    """Build gidx (wrapped int16), dstoff/onorm column-packed arrays for core c."""
    nbanks = st["nbanks"]
    eproc = st["eproc"]
    s_t, dl, grp = st["per_core"][c]
    cnt = st["counts"][c]

    gidx = np.zeros(eproc, np.int64)
    dstoff = np.full(eproc, -1.0, np.float32)
    onorm = np.zeros(eproc, np.float32)

    gstart = np.zeros(nblk * nbanks + 1, np.int64)
    np.cumsum(cnt, out=gstart[1:])
    for g, p0 in st["grp_pos"].items():
        ne = int(cnt[g])
        if ne == 0:
            continue
        e0 = int(gstart[g])
        blk = g // nbanks
        bank = g % nbanks
        sl = slice(p0, p0 + ne)
        gidx[sl] = s_t[e0:e0 + ne] - bank * BANKW
        dstoff[sl] = (dl[e0:e0 + ne] - blk * 128).astype(np.float32)
        onorm[sl] = out_norm_t[s_t[e0:e0 + ne]]

    gw = gidx.astype(np.int16).reshape(-1, 16).T          # [16, eproc/16]
    gw = np.ascontiguousarray(np.tile(gw, (8, 1)))        # [128, eproc/16]
    dstoff_p = np.ascontiguousarray(dstoff.reshape(-1, 128).T)  # [128, nch]
    onorm_p = np.ascontiguousarray(onorm.reshape(-1, 128).T)
    return gw, dstoff_p, onorm_p


def _build_program(st, nblk, slice_rows, dfeat, hfeat, pfeat, ncls, with_b1, with_b2, with_bc,
                   debug_dumps=False):
    import concourse.bacc as bacc
    import concourse.mybir as mybir
    import concourse.tile as tile

    f32 = mybir.dt.float32
    i16 = mybir.dt.int16
    Alu = mybir.AluOpType
    Act = mybir.ActivationFunctionType

    import os
    ablate = set((os.environ.get("KERNEL_ABLATE") or "").split(","))
    use_f16 = os.environ.get("KERNEL_DTYPE", "f16") == "f16"
    fpay = mybir.dt.float16 if use_f16 else f32

    nsb, nbanks, ntab = st["nsb"], st["nbanks"], st["ntab"]
    nch_tot, eproc = st["nch_tot"], st["eproc"]
    calls, chunk_meta, nchunks = st["calls"], st["chunk_meta"], st["nchunks"]
    cmax = max(nc_ for (_, _, _, nc_) in calls)
    rg = [list(range(NCORES))]

    nc = bacc.Bacc("TRN2", target_bir_lowering=False, debug=False, num_devices=NCORES)

    # ---- I/O ----
    h_t = nc.dram_tensor("h_t", [ntab, dfeat], fpay, kind="ExternalInput")
    gidx_in = nc.dram_tensor("gidx", [128, eproc // 16], i16, kind="ExternalInput")
    dstoff_in = nc.dram_tensor("dstoff", [128, nch_tot], f32, kind="ExternalInput")
    onorm_in = nc.dram_tensor("onorm", [128, nch_tot], f32, kind="ExternalInput")
    sio1_in = nc.dram_tensor("sio1", [128, nblk], f32, kind="ExternalInput")
    sin2_in = nc.dram_tensor("sin2", [128, nblk], f32, kind="ExternalInput")
    iota_in = nc.dram_tensor("iota_b", [128, 128], fpay, kind="ExternalInput")
    w1_in = nc.dram_tensor("w1", [dfeat, hfeat], f32, kind="ExternalInput")
    w2_in = nc.dram_tensor("w2", [hfeat, hfeat], f32, kind="ExternalInput")
    wca_in = nc.dram_tensor("wca", [hfeat, ncls], f32, kind="ExternalInput")
    wcb_in = nc.dram_tensor("wcb", [pfeat, ncls], f32, kind="ExternalInput")
    permt_in = nc.dram_tensor("permt", [pfeat, NUM_GRAPHS], f32, kind="ExternalInput")
    pind_in = nc.dram_tensor("pind", [slice_rows, NUM_GRAPHS], f32, kind="ExternalInput")
    b1_in = nc.dram_tensor("b1b", [128, hfeat], f32, kind="ExternalInput") if with_b1 else None
    b2_in = nc.dram_tensor("b2b", [128, hfeat], f32, kind="ExternalInput") if with_b2 else None
    bc_in = nc.dram_tensor("bcc", [128, 1], f32, kind="ExternalInput") if with_bc else None
    out_ext = nc.dram_tensor("outT", [ncls, NUM_GRAPHS], f32, kind="ExternalOutput")

    dbg_x1s = dbg_t2 = dbg_hg = None
    if debug_dumps:
        dbg_x1s = nc.dram_tensor("dbg_x1s", [slice_rows, hfeat], fpay, kind="ExternalOutput")
        dbg_t2 = nc.dram_tensor("dbg_t2", [ntab, hfeat], fpay, kind="ExternalOutput")
        dbg_hg = nc.dram_tensor("dbg_hg", [hfeat, NUM_GRAPHS], f32, kind="ExternalOutput")

    # ---- internal DRAM ----
    x1s = nc.dram_tensor("x1s", [slice_rows, hfeat], fpay)                    # AG input
    t2 = nc.dram_tensor("t2", [ntab, hfeat], fpay, addr_space="Shared")       # AG output
    ar_in = nc.dram_tensor("ar_in", [hfeat, NUM_GRAPHS], f32)
    ar_out = nc.dram_tensor("ar_out", [hfeat, NUM_GRAPHS], f32, addr_space="Shared")

    x1s_v = x1s[:].rearrange("(c p) d -> p c d", p=128)   # [128, nblk, hfeat]
    pind_v = pind_in[:].rearrange("(c p) g -> p c g", p=128)

    with tile.TileContext(nc) as tc:
        with (
            tc.tile_pool(name="const", bufs=1) as cpool,
            tc.tile_pool(name="payload", bufs=3) as ppool,
            tc.tile_pool(name="ind", bufs=8) as ipool,
            tc.tile_pool(name="epi", bufs=3) as epool,
            tc.tile_pool(name="accum", bufs=5, space="PSUM") as apsum,
            tc.tile_pool(name="ypsum", bufs=2, space="PSUM") as ypsum,
            tc.tile_pool(name="gpsum", bufs=1, space="PSUM") as gpsum,
        ):
            # ---- constants ----
            gidx_t = cpool.tile([128, eproc // 16], i16)
            dstoff_t = cpool.tile([128, nch_tot], f32)
            onorm_t = cpool.tile([128, nch_tot], f32)
            sio1_t = cpool.tile([128, nblk], f32)
            sin2_t = cpool.tile([128, nblk], f32)
            iota_t = cpool.tile([128, 128], fpay)
            w1_t = cpool.tile([dfeat, hfeat], f32)
            w2_t = cpool.tile([hfeat, hfeat], f32)
            wca_t = cpool.tile([hfeat, ncls], f32)
            wcb_t = cpool.tile([pfeat, ncls], f32)
            permt_t = cpool.tile([pfeat, NUM_GRAPHS], f32)
            for t_, i_ in [(gidx_t, gidx_in), (dstoff_t, dstoff_in), (onorm_t, onorm_in),
                           (sio1_t, sio1_in), (sin2_t, sin2_in), (iota_t, iota_in),
                           (w1_t, w1_in), (w2_t, w2_in), (wca_t, wca_in),
                           (wcb_t, wcb_in), (permt_t, permt_in)]:
                nc.sync.dma_start(t_[:], i_[:])
            b1_t = b2_t = bc_t = None
            if with_b1:
                b1_t = cpool.tile([128, hfeat], f32)
                nc.sync.dma_start(b1_t[:], b1_in[:])
            if with_b2:
                b2_t = cpool.tile([128, hfeat], f32)
                nc.sync.dma_start(b2_t[:], b2_in[:])
            if with_bc:
                bc_t = cpool.tile([128, 1], f32)
                nc.sync.dma_start(bc_t[:], bc_in[:])

            hg_ps = gpsum.tile([hfeat, NUM_GRAPHS], f32)

            def edge_layer(layer):
                """One GCN layer: gather + indicator matmuls + epilogue."""
                table = h_t if layer == 1 else t2
                nf = dfeat if layer == 1 else hfeat
                # first/last chunk bookkeeping per blk
                blk_total = np.zeros(nblk, np.int64)
                for (blk, bank) in chunk_meta:
                    blk_total[blk] += 1
                blk_seen = np.zeros(nblk, np.int64)

                # calls grouped by sb
                sb_calls = {}
                for call in calls:
                    sb_calls.setdefault(call[0], []).append(call)

                for sb in range(nsb):
                    blks = list(range(sb * SBB, min((sb + 1) * SBB, nblk)))
                    # one PSUM tile per block: a bank must host only one
                    # accumulation group at a time (start=True resets the
                    # whole bank's has_written state)
                    accs = [apsum.tile([nf, 128], f32, tag="acc", name=f"acc_{layer}_{sb}_{i}")
                            for i in range(len(blks))]
                    for b4, blk in enumerate(blks):
                        if blk_total[blk] == 0:
                            nc.vector.memset(accs[b4][:], 0.0)
                    for (_, bank, k0, nch) in sb_calls.get(sb, []):
                        pt = ppool.tile([128, cmax, 128], fpay, tag="pay")
                        lo = bank * BANKW
                        hi = min(lo + BANKW, ntab)
                        if "nogather" not in ablate:
                            nc.gpsimd.dma_gather(
                                pt[:, :nch, :], table[lo:hi, :],
                                gidx_t[:, 8 * k0: 8 * (k0 + nch)],
                                128 * nch, 128 * nch, nf, single_packet=False)
                        else:
                            nc.any.memset(pt[:, 0, :1], 0.0)
                        for j in range(nch):
                            k = k0 + j
                            blk = chunk_meta[k][0]
                            b4 = blk - sb * SBB
                            ind = ipool.tile([128, 128], fpay, tag="ind")
                            if "noind" in ablate:
                                nc.any.memset(ind[:, :1], 0.0)
                            elif True:
                                if layer == 1:
                                    nc.vector.tensor_scalar(
                                        ind[:], iota_t[:],
                                        dstoff_t[:, k:k + 1], onorm_t[:, k:k + 1],
                                        Alu.is_equal, Alu.mult)
                                else:
                                    nc.vector.tensor_scalar(
                                        ind[:], iota_t[:],
                                        dstoff_t[:, k:k + 1], None, Alu.is_equal)
                            blk_seen[blk] += 1
                            if "nomm" in ablate and blk_seen[blk] not in (1, blk_total[blk]):
                                continue
                            nc.tensor.matmul(
                                accs[b4][:], pt[:, j, :], ind[:],
                                start=bool(blk_seen[blk] == 1),
                                stop=bool(blk_seen[blk] == blk_total[blk]),
                                skip_group_check=True)
                    # ---- epilogue for this superblock ----
                    aggT = epool.tile([nf, SBB * 128], f32, tag="aggT")
                    aggTv = aggT[:].rearrange("p (b n) -> p b n", b=SBB)
                    for b4 in range(len(blks)):
                        nc.vector.tensor_copy(aggTv[:, b4, :], accs[b4][:])
                    if layer == 1:
                        xt = epool.tile([128, SBB, hfeat], fpay, tag="xt")
                        for b4, blk in enumerate(blks):
                            y = ypsum.tile([128, hfeat], f32, tag="y")
                            nc.tensor.matmul(y[:], aggTv[:, b4, :], w1_t[:],
                                             start=True, stop=True, skip_group_check=True)
                            if with_b1:
                                # x = relu(in_norm*y + b1) * out_norm
                                # (sio1 holds out_norm in this mode, sin2 = in_norm)
                                v = epool.tile([128, hfeat], f32, tag="v")
                                nc.vector.tensor_scalar(v[:], y[:], sin2_t[:, blk:blk + 1],
                                                        None, Alu.mult)
                                nc.vector.tensor_tensor(v[:], v[:], b1_t[:], Alu.add)
                                nc.scalar.activation(xt[:, b4, :], v[:], Act.Relu,
                                                     scale=sio1_t[:, blk:blk + 1])
                            else:
                                # x = relu(y) * in*out = relu(y * (in*out))
                                nc.scalar.activation(xt[:, b4, :], y[:], Act.Relu,
                                                     scale=sio1_t[:, blk:blk + 1])
                        nc.sync.dma_start(x1s_v[:, sb * SBB: sb * SBB + len(blks), :],
                                          xt[:, :len(blks), :])
                    else:
                        for b4, blk in enumerate(blks):
                            y = ypsum.tile([128, hfeat], f32, tag="y")
                            nc.tensor.matmul(y[:], aggTv[:, b4, :], w2_t[:],
                                             start=True, stop=True, skip_group_check=True)
                            x2 = epool.tile([128, hfeat], f32, tag="x2")
                            nc.scalar.activation(x2[:], y[:], Act.Copy,
                                                 scale=sin2_t[:, blk:blk + 1])
                            if with_b2:
                                nc.vector.tensor_tensor(x2[:], x2[:], b2_t[:], Alu.add)
                            pi = epool.tile([128, NUM_GRAPHS], f32, tag="pi")
                            nc.sync.dma_start(pi[:], pind_v[:, blk, :])
                            nc.tensor.matmul(hg_ps[:], x2[:], pi[:],
                                             start=bool(blk == 0),
                                             stop=bool(blk == nblk - 1),
                                             skip_group_check=True)

            # NOTE on layer-1 bias: for with_b1 the sio1 scale on `u` above is
            # unused scratch; the correct path uses sin2 (=in_norm) + b1 + relu
            # with sio... see host: sio1 = in*out for no-bias, out-only for bias.

            if "noedge1" not in ablate:
                edge_layer(1)
            if "nocoll" not in ablate:
                nc.gpsimd.collective_compute(
                    "AllGather", mybir.AluOpType.bypass, replica_groups=rg,
                    ins=[x1s[:]], outs=[t2[:]])
            if "noedge2" not in ablate:
                edge_layer(2)
            if debug_dumps:
                nc.sync.dma_start(dbg_x1s[:], x1s[:])
                nc.sync.dma_start(dbg_t2[:], t2[:])

            # ---- pooled sums all-reduce + head ----
            hg_sb = epool.tile([hfeat, NUM_GRAPHS], f32, tag="hg")
            if "noedge2" not in ablate:
                nc.vector.tensor_copy(hg_sb[:], hg_ps[:])
            nc.sync.dma_start(ar_in[:], hg_sb[:])
            if "nocoll" not in ablate:
                nc.gpsimd.collective_compute(
                    "AllReduce", mybir.AluOpType.add, replica_groups=rg,
                    ins=[ar_in[:]], outs=[ar_out[:]])
            hg_all = epool.tile([hfeat, NUM_GRAPHS], f32, tag="hga")
            nc.sync.dma_start(hg_all[:], ar_out[:])
            if debug_dumps:
                nc.sync.dma_start(dbg_hg[:], ar_in[:])

            o_ps = ypsum.tile([128, NUM_GRAPHS], f32, tag="y")
            nc.tensor.matmul(o_ps[:ncls, :], wca_t[:], hg_all[:],
                             start=True, stop=False, skip_group_check=True)
            nc.tensor.matmul(o_ps[:ncls, :], wcb_t[:], permt_t[:],
                             start=False, stop=True, skip_group_check=True)
            ob = epool.tile([128, NUM_GRAPHS], f32, tag="ob")
            if with_bc:
                nc.vector.tensor_scalar(ob[:ncls, :], o_ps[:ncls, :],
                                        bc_t[:ncls, :], None, Alu.add)
            else:
                nc.vector.tensor_copy(ob[:ncls, :], o_ps[:ncls, :])
            nc.sync.dma_start(out_ext[:], ob[:ncls, :])

    nc.compile()
    return nc


def kernel(h, perm_features, W1, b1, W2, b2, Wc, bc, src, dst, graph_ids):
    from concourse.bass_utils import run_bass_kernel_spmd

    h = np.ascontiguousarray(np.asarray(h, dtype=np.float32))
    perm_features = np.asarray(perm_features, dtype=np.float32)
    W1 = np.asarray(W1, dtype=np.float32)
    W2 = np.asarray(W2, dtype=np.float32)
    Wc = np.asarray(Wc, dtype=np.float32)
    b1 = np.asarray(b1, dtype=np.float32)
    b2 = np.asarray(b2, dtype=np.float32)
    bc = np.asarray(bc, dtype=np.float32)
    src = np.asarray(src).astype(np.int64)
    dst = np.asarray(dst).astype(np.int64)
    graph_ids = np.asarray(graph_ids).astype(np.int64)

    import os
    use_f16 = os.environ.get("KERNEL_DTYPE", "f16") == "f16"
    pay_np = np.float16 if use_f16 else np.float32

    n_nodes, dfeat = h.shape
    hfeat = W1.shape[1]
    pfeat = perm_features.shape[1]
    ncls = Wc.shape[1]
    n_edges = src.shape[0]
    own = _ceil(n_nodes, NCORES)
    nblk = _ceil(own, 128)
    slice_rows = nblk * 128
    ntab = NCORES * slice_rows

    # ---- degrees / norms (host: index histograms, part of graph partitioning) ----
    out_deg = np.bincount(src, minlength=n_nodes).astype(np.float64)
    in_deg = np.bincount(dst, minlength=n_nodes).astype(np.float64)
    out_norm = (1.0 / np.sqrt(np.maximum(out_deg, 1.0))).astype(np.float32)
    in_norm = (1.0 / np.sqrt(np.maximum(in_deg, 1.0))).astype(np.float32)

    # node table in t-order (core-major with per-core pad to slice_rows)
    h_tab = np.zeros((ntab, dfeat), pay_np)
    out_norm_t = np.zeros(ntab, np.float32)
    in_norm_t = np.ones(ntab, np.float32)
    for c in range(NCORES):
        lo, hi = c * own, min((c + 1) * own, n_nodes)
        h_tab[c * slice_rows: c * slice_rows + (hi - lo)] = h[lo:hi]
        out_norm_t[c * slice_rows: c * slice_rows + (hi - lo)] = out_norm[lo:hi]
        in_norm_t[c * slice_rows: c * slice_rows + (hi - lo)] = in_norm[lo:hi]

    st = _build_structure(src, dst, n_nodes, own, slice_rows, nblk)

    with_b1 = bool(np.any(b1))
    with_b2 = bool(np.any(b2))
    with_bc = bool(np.any(bc))

    # per-block scale columns (per core)
    counts = np.bincount(graph_ids, minlength=NUM_GRAPHS).astype(np.float64)
    invcnt = (1.0 / np.maximum(counts, 1.0)).astype(np.float32)

    iota_b = np.tile(np.arange(128, dtype=pay_np), (128, 1))
    permt = np.ascontiguousarray(perm_features.T)

    in_maps = []
    for c in range(NCORES):
        gw, dstoff_p, onorm_p = _pack_core(st, c, out_norm_t, nblk)
        sl = slice(c * slice_rows, (c + 1) * slice_rows)
        s_in = in_norm_t[sl].reshape(nblk, 128).T.copy()     # [128, nblk]
        s_out = np.where(out_norm_t[sl] > 0, out_norm_t[sl], 1.0) \
            .reshape(nblk, 128).T.copy()
        sio1 = np.ascontiguousarray(s_in * s_out) if not with_b1 else np.ascontiguousarray(s_out)
        # pooling indicator with 1/count folded in
        gsl = graph_ids[c * own: min((c + 1) * own, n_nodes)]
        pind = np.zeros((slice_rows, NUM_GRAPHS), np.float32)
        pind[np.arange(gsl.shape[0]), gsl] = invcnt[gsl]
        m = {
            "h_t": h_tab, "gidx": gw, "dstoff": dstoff_p, "onorm": onorm_p,
            "sio1": sio1, "sin2": np.ascontiguousarray(s_in), "iota_b": iota_b,
            "w1": W1, "w2": W2, "wca": np.ascontiguousarray(Wc[:hfeat]),
            "wcb": np.ascontiguousarray(Wc[hfeat:]), "permt": permt,
            "pind": pind,
        }
        if with_b1:
            m["b1b"] = np.tile(b1, (128, 1))
        if with_b2:
            m["b2b"] = np.tile(b2, (128, 1))
        if with_bc:
            m["bcc"] = np.pad(bc, (0, 128 - ncls)).reshape(128, 1)
        in_maps.append(m)

    import os
    nc = _build_program(st, nblk, slice_rows, dfeat, hfeat, pfeat, ncls,
                        with_b1, with_b2, with_bc,
                        debug_dumps=os.environ.get("KERNEL_DEBUG") == "1")
    if os.environ.get("KERNEL_TIME") == "1":
        out0, times = _timed_run(nc, in_maps)
        kernel.last_times = times
        return np.ascontiguousarray(out0["outT"].T)
    r = run_bass_kernel_spmd(nc, in_maps, list(range(NCORES)))
    kernel.last_result = r
    return np.ascontiguousarray(r.results[0]["outT"].T)


def _timed_run(nc, in_maps, iters=6):
    """Run the program on 8 cores with a persistent jit (no donation) and
    time repeated executions."""
    import time

    import jax
    import numpy as np
    from jax.experimental.shard_map import shard_map
    from jax.sharding import Mesh, PartitionSpec

    from concourse import bass2jax, mybir

    bass2jax.install_neuronx_cc_hook()
    n_cores = NCORES
    partition_name = nc.partition_id_tensor.name if nc.partition_id_tensor else None
    in_names, out_names, out_avals, zero_outs = [], [], [], []
    for alloc in nc.m.functions[0].allocations:
        if not isinstance(alloc, mybir.MemoryLocationSet):
            continue
        name = alloc.memorylocations[0].name
        if alloc.kind == "ExternalInput":
            if name != partition_name:
                in_names.append(name)
        elif alloc.kind == "ExternalOutput":
            out_names.append(name)
            shape = tuple(alloc.tensor_shape)
            dtype = mybir.dt.np(alloc.dtype)
            out_avals.append(jax.core.ShapedArray(shape, dtype))
            zero_outs.append(np.zeros(shape, dtype))
    n_params = len(in_names)
    all_in_names = in_names + out_names + ([partition_name] if partition_name else [])

    def _body(*args):
        operands = list(args)
        if partition_name is not None:
            operands.append(bass2jax.partition_id_tensor())
        outs = bass2jax._bass_exec_p.bind(
            *operands,
            out_avals=tuple(out_avals),
            in_names=tuple(all_in_names),
            out_names=tuple(out_names),
            lowering_input_output_aliases=(),
            sim_require_finite=True,
            sim_require_nnan=True,
            nc=nc,
        )
        return tuple(outs)

    devices = jax.devices()[:n_cores]
    mesh = Mesh(np.asarray(devices), ("core",))
    in_specs = (PartitionSpec("core"),) * (n_params + len(out_names))
    out_specs = (PartitionSpec("core"),) * len(out_names)
    sharded = jax.jit(shard_map(_body, mesh=mesh, in_specs=in_specs,
                                out_specs=out_specs, check_rep=False),
                      keep_unused=True)
    concat_in = [np.concatenate([np.asarray(in_maps[c][nm]) for c in range(n_cores)], axis=0)
                 for nm in in_names]
    concat_zeros = [np.zeros((n_cores * z.shape[0], *z.shape[1:]), z.dtype)
                    for z in zero_outs]
    # stage on device once, already sharded across the 8 cores so the timed
    # loop doesn't pay a host->device or device->device reshard per call
    from jax.sharding import NamedSharding
    sh = NamedSharding(mesh, PartitionSpec("core"))
    dev_in = [jax.device_put(a, sh) for a in concat_in]
    dev_z = [jax.device_put(a, sh) for a in concat_zeros]
    out = sharded(*dev_in, *dev_z)
    jax.block_until_ready(out)
    times = []
    for _ in range(iters):
        t0 = time.perf_counter()
        out = sharded(*dev_in, *dev_z)
        jax.block_until_ready(out)
        times.append(time.perf_counter() - t0)
    out0 = {nm: np.asarray(out[i]).reshape(n_cores, *out_avals[i].shape)[0]
            for i, nm in enumerate(out_names)}
    return out0, times
